# revision 1
# baseline (speedup 1.0000x reference)
"""Causal attention head (k==v source quirk) on 8 trn2 NeuronCores.

Math per batch b:
  q = x[b] @ WQ ; kv = x[b] @ WV        (k and v are the SAME projection)
  S = q @ kv^T ; causal mask ; P = softmax(S) (no sqrt(d) scale)
  out[b] = P @ kv

Sharding: core = (b, h), h in {0,1}. Balanced causal split of the 8
512-row query blocks of batch b: h=0 gets blocks [0,2,5,7], h=1 gets
[1,3,4,6]. Keys are column-permuted per core (host side) so that every
core runs the IDENTICAL program: chunk c (c=0..3) attends to the first
1024*(c+1) keys of its permuted key buffer; the diagonal (own) block
always sits at buffer slot 2c+1 and the slot 2c block is either fully
valid or fully dead, selected by a per-core multiplicative flag (1/0)
applied to the exp'd probabilities.

On-chip: transposed-score form S^T[s, q] (s on partitions) makes QK^T,
exp, and P^T@V all transpose-free. Softmax needs no max-subtraction
(max logit ~61, row sums ~4e26 << fp32 max): exp directly, denominator
accumulated via a ones-column appended to V, final divide on host
during unsharding.

Built with bacc.Bacc + finalize(): its generate_event_semaphores pass
splits semaphore waits to satisfy this walrus's one-wait-per-instruction
limit. Instruction order still minimizes cross-engine waits (qt evac
last per piece, exp always the last writer before the PV matmul).

QK row-tiling: the QK matmuls contract over only K=64, so consecutive
score tiles are issued into disjoint PE row groups (even tiles rows
0-63, odd tiles rows 64-127 via tile_position=(64,0)) and run
concurrently on hardware. The odd-tile operands are duplicates of
K^T/Q^T on partitions 64-127, filled by CROSS-TENSOR SBUF->SBUF DMAs —
a same-tensor overlapping DMA (upper half <- lower half of one tile)
crashes the device with NRT_EXEC_UNIT_UNRECOVERABLE; separate
source/destination tensors work.
"""

import os
import sys

import numpy as np

sys.path.insert(0, "/opt/trn_rl_repo")

import concourse.bass as bass
import concourse.bacc as bacc
import concourse.mybir as mybir
from concourse.bass_utils import run_bass_kernel_spmd
from concourse.tile import TileContext

P = 128
T = 4096
C = 1024
D = 64
NCTILE = C // P      # 8 contraction tiles
NCHUNK = 4           # query chunks per core (512 queries each)
QW = 512             # queries per chunk
NQ = NCHUNK * QW     # 2048 queries per core

# per-core block orders (positions in x of each 512-col block of the key buffer)
KEY_ORDER = {0: [1, 0, 3, 2, 4, 5, 6, 7], 1: [0, 1, 2, 3, 5, 4, 7, 6]}
# query block (= diagonal block) of chunk c is key-buffer slot 2c+1
Q_BLOCKS = {0: [0, 2, 5, 7], 1: [1, 3, 4, 6]}
# multiplicative flag for key-buffer slot 2c in chunk c: 1.0 = valid, 0.0 = dead
FLAGS = {0: [0.0, 0.0, 1.0, 1.0], 1: [1.0, 1.0, 0.0, 0.0]}

F32 = mybir.dt.float32


def build_nc():
    nc = bacc.Bacc("TRN2")
    xt = nc.dram_tensor("xt", [C, T], F32, kind="ExternalInput")
    wq = nc.dram_tensor("wq", [C, D], F32, kind="ExternalInput")
    wv = nc.dram_tensor("wv", [C, D], F32, kind="ExternalInput")
    flags = nc.dram_tensor("flags", [P, NCHUNK], F32, kind="ExternalInput")
    o = nc.dram_tensor("o", [D + 1, NQ], F32, kind="ExternalOutput")

    with TileContext(nc) as tc:
        with (
            tc.tile_pool(name="persist", bufs=1) as persist,
            tc.tile_pool(name="xpool", bufs=3) as xpool,
            tc.tile_pool(name="ppool", bufs=4) as ppool,
            tc.tile_pool(name="pproj", bufs=1, space="PSUM") as pproj,
            tc.tile_pool(name="pattn", bufs=4, space="PSUM") as pattn,
        ):
            # --- constants (gpsimd-built / DMA'd) ---
            ident = persist.tile([P, P], F32, tag="ident", name="ident")
            nc.vector.memset(ident, 1.0)
            nc.gpsimd.affine_select(
                out=ident, in_=ident, pattern=[[-1, P]],
                compare_op=mybir.AluOpType.is_equal, fill=0.0,
                base=0, channel_multiplier=1,
            )
            # stacked weights: cols 0-63 = WV, 64-127 = WQ per c-tile, so one
            # matmul emits KV^T (out rows 0-63) and Q^T (rows 64-127) together
            wvq = persist.tile([P, NCTILE, 2 * D], F32, tag="wvq", name="wvq")
            nc.sync.dma_start(
                wvq[:, :, 0:D], wv[:, :].rearrange("(j p) d -> p j d", p=P)
            )
            nc.sync.dma_start(
                wvq[:, :, D : 2 * D], wq[:, :].rearrange("(j p) d -> p j d", p=P)
            )
            flg0 = persist.tile([P, NCHUNK], F32, tag="flg0", name="flg0")
            nc.sync.dma_start(flg0, flags[:, :])
            flg = persist.tile([P, NCHUNK], F32, tag="flg", name="flg")
            nc.vector.tensor_copy(flg, flg0)  # seed DVE clock with the flags DMA

            # --- persistent SBUF state ---
            kt = persist.tile([D, T], F32, tag="kt", name="kt")          # KV^T
            qt = persist.tile([D, NQ], F32, tag="qt", name="qt")         # Q^T
            vp = persist.tile([P, T // P, D + 1], F32, tag="vp", name="vp")  # V'
            # duplicates of K^T / Q^T living on partitions 64-127 (filled by
            # cross-tensor SBUF->SBUF DMA) so odd score tiles can run in PE
            # rows 64-127 concurrently with even tiles in rows 0-63
            ktw = persist.tile([P, T], F32, tag="ktw", name="ktw")
            qtw = persist.tile([P, NQ], F32, tag="qtw", name="qtw")
            o_sb = persist.tile([D + 1, NQ], F32, tag="o_sb", name="o_sb")

            for p in range(NCHUNK):
                # ---- load xt piece p: cols [1024p, 1024(p+1)), all 8 c-tiles
                # in ONE DMA (keeps every DMA on its own lane, <=1 wait) ----
                xtp = xpool.tile([P, NCTILE, 1024], F32, tag="xtp", name=f"xtp_{p}")
                for e8 in range(8):
                    for ch in range(2):
                        nc.sync.dma_start(
                            xtp[:, e8 : e8 + 1, 512 * ch : 512 * ch + 512],
                            xt[
                                128 * e8 : 128 * e8 + 128,
                                1024 * p + 512 * ch : 1024 * p + 512 * ch + 512,
                            ].rearrange("(j p) c -> p j c", p=P),
                        )
                # ---- projections for this piece ----
                kv_lo = pproj.tile([D, 512], F32, tag="kv_lo", name=f"kv_lo_{p}")
                combo = pproj.tile([P, 512], F32, tag="combo", name=f"combo_{p}")
                for j in range(NCTILE):
                    st_, sp_ = (j == 0), (j == NCTILE - 1)
                    nc.tensor.matmul(
                        kv_lo, wvq[:, j, 0:D], xtp[:, j, 0:512], start=st_, stop=sp_
                    )
                    nc.tensor.matmul(
                        combo, wvq[:, j, :], xtp[:, j, 512:1024], start=st_, stop=sp_
                    )
                nc.vector.tensor_copy(kt[:, 1024 * p : 1024 * p + 512], kv_lo)
                nc.vector.tensor_copy(
                    kt[:, 1024 * p + 512 : 1024 * (p + 1)], combo[0:D, :]
                )
                nc.sync.dma_start(
                    ktw[D : 2 * D, 1024 * p : 1024 * (p + 1)],
                    kt[:, 1024 * p : 1024 * (p + 1)],
                )
                # ---- V' tiles (transpose KV^T back to natural + ones col) ----
                for tt in range(8):
                    t = 8 * p + tt
                    vt_ps = pattn.tile([P, D], F32, tag="st", name=f"vt_{t}")
                    nc.tensor.transpose(
                        vt_ps, kt[:, P * t : P * (t + 1)], ident[0:D, 0:D]
                    )
                    nc.vector.tensor_copy(vp[:, t, 0:D], vt_ps)
                    nc.vector.memset(vp[:, t, D : D + 1], 1.0)
                # qt evac LAST: the first QK's DVE wait then covers all of the above
                nc.vector.tensor_copy(
                    qtw[D : 2 * D, QW * p : QW * (p + 1)], combo[D : 2 * D, :]
                )
                nc.sync.dma_start(
                    qt[:, QW * p : QW * (p + 1)],
                    qtw[D : 2 * D, QW * p : QW * (p + 1)],
                )
                # ---- attention for chunk p ----
                out_ps = pattn.tile([D + 1, QW], F32, tag="out", name=f"out_{p}", bufs=2)
                n_st = 8 * (p + 1)
                st_tiles = []
                LOOKAHEAD = 2
                for t in range(n_st):
                    st_ps = pattn.tile([P, QW], F32, tag="st", name=f"st_{p}_{t}")
                    if t % 2 == 0:
                        nc.tensor.matmul(
                            st_ps, kt[:, P * t : P * (t + 1)],
                            qt[:, QW * p : QW * (p + 1)], start=True, stop=True,
                        )
                    else:
                        nc.tensor.matmul(
                            st_ps, ktw[D : 2 * D, P * t : P * (t + 1)],
                            qtw[D : 2 * D, QW * p : QW * (p + 1)],
                            start=True, stop=True, tile_position=(D, 0),
                        )
                    st_tiles.append(st_ps)
                    # process tile t-LOOKAHEAD while QK of t runs (keeps PE dense)
                    if t >= LOOKAHEAD:
                        _attn_tail(nc, ppool, flg, vp, out_ps, st_tiles,
                                   p, t - LOOKAHEAD, n_st)
                for t in range(max(0, n_st - LOOKAHEAD), n_st):
                    _attn_tail(nc, ppool, flg, vp, out_ps, st_tiles, p, t, n_st)
                nc.vector.tensor_copy(o_sb[:, QW * p : QW * (p + 1)], out_ps)
            nc.sync.dma_start(o[:, :], o_sb)
    if not nc.is_finalized():
        nc.finalize()
    return nc


def _attn_tail(nc, ppool, flg, vp, out_ps, st_tiles, p, t, n_st):
    """exp + post-exp mask (gpsimd) + PV-accumulate for score tile t of chunk p."""
    pt = ppool.tile([P, QW], F32, tag="pt", name=f"pt_{p}_{t}")
    nc.scalar.activation(pt, st_tiles[t], mybir.ActivationFunctionType.Exp)
    if 8 * p <= t < 8 * p + 4:
        # key-buffer slot 2p: fully valid or fully dead, per-core 1/0 flag
        nc.gpsimd.tensor_scalar_mul(pt, pt, flg[:, p : p + 1])
    elif t >= 8 * p + 4:
        # diagonal block (slot 2p+1): zero out entries above the causal line
        k = t - (8 * p + 4)
        nc.gpsimd.affine_select(
            out=pt, in_=pt, pattern=[[1, QW]],
            compare_op=mybir.AluOpType.is_ge, fill=0.0,
            base=-(P * k), channel_multiplier=-1,
        )
    nc.tensor.matmul(
        out_ps, vp[:, t, :], pt, start=(t == 0), stop=(t == n_st - 1)
    )


_CACHED_NC = None


def kernel(**inputs):
    global _CACHED_NC
    x = np.ascontiguousarray(np.asarray(inputs["x"], dtype=np.float32))
    WQ = np.ascontiguousarray(np.asarray(inputs["WQ"], dtype=np.float32))
    WV = np.ascontiguousarray(np.asarray(inputs["WV"], dtype=np.float32))
    B = x.shape[0]

    if _CACHED_NC is None:
        _CACHED_NC = build_nc()
    nc = _CACHED_NC

    in_maps = []
    for core in range(8):
        b, h = divmod(core, 2)
        xtb = x[b].T  # [C, T]
        cols = np.concatenate(
            [np.arange(512 * j, 512 * (j + 1)) for j in KEY_ORDER[h]]
        )
        in_maps.append(
            {
                "xt": np.ascontiguousarray(xtb[:, cols]),
                "wq": WQ,
                "wv": WV,
                "flags": np.broadcast_to(
                    np.asarray(FLAGS[h], np.float32), (P, NCHUNK)
                ).copy(),
            }
        )

    trace = os.environ.get("KERNEL_TRACE", "0") == "1"
    res = run_bass_kernel_spmd(nc, in_maps, core_ids=list(range(8)), trace=trace)
    kernel._last_results = res

    out = np.empty((B, T, D), dtype=np.float32)
    for core in range(8):
        b, h = divmod(core, 2)
        o = res.results[core]["o"]  # [65, 2048]
        full = (o[:D, :] / o[D, :]).T  # [2048, 64]
        for c, j in enumerate(Q_BLOCKS[h]):
            out[b, 512 * j : 512 * (j + 1)] = full[QW * c : QW * (c + 1)]
    return out



# revision 50
# speedup vs baseline: 1.7004x; 1.7004x over previous
"""Causal attention head (k==v source quirk) on 8 trn2 NeuronCores.

Math per batch b:
  q = x[b] @ WQ ; kv = x[b] @ WV        (k and v are the SAME projection)
  S = q @ kv^T ; causal mask ; P = softmax(S) (no sqrt(d) scale)
  out[b] = P @ kv

Sharding: core = (b, h), h in {0,1}. Balanced causal split of the 8
512-row query blocks of batch b: h=0 gets blocks [0,2,5,7], h=1 gets
[1,3,4,6]. Keys are column-permuted per core (host side) so that every
core runs the IDENTICAL program: chunk c (c=0..3) attends to the first
1024*(c+1) keys of its permuted key buffer; the diagonal (own) block
always sits at buffer slot 2c+1 and the slot 2c block is either fully
valid or fully dead, selected by a per-core additive bias (0 / -1e30)
folded into the exp activation.

Timeline-model cost of a matmul is moving-operand columns times a
per-dtype cycles/row (fp32=4, fp32r=1 when >=256 cols); contraction
depth (<=128) is free. Hence:
  - projections run "flipped": stationary = x c-tile [128c, 128t],
    moving = W columns -> 4x fewer columns, and kv lands directly in
    natural [t, d] layout (vp), no V transposes.
  - QK runs as fp32r hi/lo limbs (11-bit each) in TWO matmuls per
    score tile: main = k_hi . q_hi (K=64), cross = k_hi . q_lo +
    k_lo . q_hi STACKED into one K=128 matmul (kts rows 0-63 = hi,
    64-127 = lo; qts rows 0-63 = q_lo, 64-127 = q_hi). Near-fp32
    logits at 2 column-passes instead of fp32's 4.
  - PV runs "flipped": stationary = P [128s, 128q] quarter-tiles,
    moving = V' [128s, 65] -> 260 cols/tile instead of 512, full fp32.
  - the diagonal causal mask is a DVE multiply with one of 4
    precomputed 0/1 mask tiles (keeps the per-tile exp->mask->PV chain
    short); dead slots die via the exp bias, so GpSimd is off the
    critical path entirely.
  - score tiles use a 3-deep lookahead (QK of tiles t..t+3 issue ahead
    of the exp/PV tail of tile t) and stages are software-pipelined
    (transposes of piece p+1 + projections of piece p+2 are emitted
    before the last attention tails of piece p) so the PE never idles
    and stays at max p-state.
Score form is transposed S^T[s, q]; softmax needs no max-subtraction
(max logit ~61); the denominator rides as a ones column appended to
V'; final divide on host in fp64 during unsharding.
"""

import os
import sys

import numpy as np

sys.path.insert(0, "/opt/trn_rl_repo")

import concourse.bass as bass
import concourse.bacc as bacc
import concourse.mybir as mybir
from concourse.bass_utils import run_bass_kernel_spmd
from concourse.tile import TileContext

P = 128
T = 4096
C = 1024
D = 64
NCTILE = C // P      # 8 contraction tiles
NCHUNK = 4           # query chunks per core (512 queries each)
QW = 512             # queries per chunk
NQ = NCHUNK * QW     # 2048 queries per core
NT = T // P          # 32 key tiles
LOOKAHEAD = 3

KEY_ORDER = {0: [1, 0, 3, 2, 4, 5, 6, 7], 1: [0, 1, 2, 3, 5, 4, 7, 6]}
Q_BLOCKS = {0: [0, 2, 5, 7], 1: [1, 3, 4, 6]}
# additive exp-bias for key-buffer slot 2c in chunk c: 0 = valid, -1e30 = dead
BIAS = {0: [-1e30, -1e30, 0.0, 0.0], 1: [0.0, 0.0, -1e30, -1e30]}

F32 = mybir.dt.float32
F32R = mybir.dt.float32r


def build_nc():
    nc = bacc.Bacc("TRN2")
    xt = nc.dram_tensor("xt", [C, T], F32, kind="ExternalInput")
    wq = nc.dram_tensor("wq", [C, D], F32, kind="ExternalInput")
    wv = nc.dram_tensor("wv", [C, D], F32, kind="ExternalInput")
    flgb = nc.dram_tensor("flgb", [P, NCHUNK], F32, kind="ExternalInput")
    o = nc.dram_tensor("o", [P, NQ // P, D + 1], F32, kind="ExternalOutput")

    with TileContext(nc) as tc:
        with (
            tc.tile_pool(name="persist", bufs=1) as persist,
            tc.tile_pool(name="xpool", bufs=2) as xpool,
            tc.tile_pool(name="qnpool", bufs=8) as qnpool,
            tc.tile_pool(name="ppool", bufs=4) as ppool,
            tc.tile_pool(name="pproj", bufs=2, space="PSUM") as pproj,
            tc.tile_pool(name="pattn", bufs=5, space="PSUM") as pattn,
            tc.tile_pool(name="pout", bufs=1, space="PSUM") as pout,
        ):
            # --- constants ---
            ident = persist.tile([P, P], F32, tag="ident", name="ident")
            nc.gpsimd.memset(ident, 1.0)
            nc.gpsimd.affine_select(
                out=ident, in_=ident, pattern=[[-1, P]],
                compare_op=mybir.AluOpType.is_equal, fill=0.0,
                base=0, channel_multiplier=1,
            )
            # 4 diagonal causal masks: dmask[k][s, c] = (c >= 128k + s)
            dmask = persist.tile([P, NCHUNK, QW], F32, tag="dmask", name="dmask")
            nc.gpsimd.memset(dmask, 1.0)
            for k in range(NCHUNK):
                nc.gpsimd.affine_select(
                    out=dmask[:, k, :], in_=dmask[:, k, :], pattern=[[1, QW]],
                    compare_op=mybir.AluOpType.is_ge, fill=0.0,
                    base=-(P * k), channel_multiplier=-1,
                )
            wvq = persist.tile([P, NCTILE, 2 * D], F32, tag="wvq", name="wvq")
            nc.sync.dma_start(
                wvq[:, :, 0:D], wv[:, :].rearrange("(j p) d -> p j d", p=P)
            )
            nc.sync.dma_start(
                wvq[:, :, D : 2 * D], wq[:, :].rearrange("(j p) d -> p j d", p=P)
            )
            flgb0 = persist.tile([P, NCHUNK], F32, tag="flgb0", name="flgb0")
            nc.sync.dma_start(flgb0, flgb[:, :])
            flg = persist.tile([P, NCHUNK], F32, tag="flg", name="flg")
            nc.vector.tensor_copy(flg, flgb0)  # seed DVE clock on the DMA

            # --- persistent SBUF state ---
            vp = persist.tile([P, NT, D + 1], F32, tag="vp", name="vp")
            nc.gpsimd.memset(vp[:, :, D : D + 1], 1.0)
            # stacked fp32r limb tensors (matmul operands must share a base
            # partition, so q_hi is stored twice):
            #   kts rows 0-63 = kv_hi, rows 64-127 = kv_lo
            #   qtm           = q_hi           (main matmul moving operand)
            #   qts rows 0-63 = q_lo, rows 64-127 = q_hi (pairs with kts)
            kts = persist.tile([P, T], F32R, tag="kts", name="kts")
            qtm = persist.tile([D, NQ], F32R, tag="qtm", name="qtm")
            qts = persist.tile([P, NQ], F32R, tag="qts", name="qts")
            o_sb = persist.tile([P, NQ // P, D + 1], F32, tag="o_sb", name="o_sb")

            xtps = [None] * NCHUNK
            qn_tiles = {}

            def stage_load(p):
                xtp = xpool.tile([P, NCTILE, 1024], F32, tag="xtp", name=f"xtp_{p}")
                xtps[p] = xtp
                for e8 in range(8):
                    for ch in range(2):
                        nc.sync.dma_start(
                            xtp[:, e8 : e8 + 1, 512 * ch : 512 * ch + 512],
                            xt[
                                128 * e8 : 128 * e8 + 128,
                                1024 * p + 512 * ch : 1024 * p + 512 * ch + 512,
                            ].rearrange("(j p) c -> p j c", p=P),
                        )

            def proj_units(p):
                """Yield one projection-chain emitter per 128-key tile."""
                xtp = xtps[p]
                for tt in range(8):
                    def emit(tt=tt):
                        t = 8 * p + tt
                        own = tt >= 4
                        width = 2 * D if own else D
                        vq_ps = pproj.tile([P, 2 * D], F32, tag="vq",
                                           name=f"vq_{t}")
                        for j in range(NCTILE):
                            nc.tensor.matmul(
                                vq_ps[:, 0:width],
                                xtp[:, j, P * tt : P * (tt + 1)],
                                wvq[:, j, 0:width],
                                start=(j == 0), stop=(j == NCTILE - 1),
                            )
                        # evacuate on ACT (activation-Copy): keeps the DVE
                        # queue free for the transpose limb ops
                        nc.scalar.activation(
                            vp[:, t, 0:D], vq_ps[:, 0:D],
                            mybir.ActivationFunctionType.Copy,
                        )
                        if own:
                            qn = qnpool.tile([P, D], F32, tag="qn",
                                             name=f"qn_{t}")
                            nc.vector.tensor_copy(qn, vq_ps[:, D : 2 * D])
                            qn_tiles[(p, tt - 4)] = qn
                    yield emit

            def transp_units(p):
                """Yield 3 transpose-batch emitters (2x kv, 1x q); the PSUM
                batch tile borrows an 'st' buffer from pattn."""
                for half in range(2):
                    def emit_k(half=half):
                        tp = pattn.tile([P, 512], F32, tag="st",
                                        name=f"ktp_{p}_{half}")
                        ktp = tp[0:D, :]
                        for k4 in range(4):
                            t = 8 * p + 4 * half + k4
                            nc.tensor.transpose(
                                ktp[:, P * k4 : P * (k4 + 1)], vp[:, t, 0:D],
                                ident,
                            )
                        lo = 1024 * p + 512 * half
                        nc.vector.tensor_copy(kts[0:D, lo : lo + 512], ktp)
                        nc.vector.tensor_tensor(
                            out=kts[D : 2 * D, lo : lo + 512],
                            in0=ktp, in1=kts[0:D, lo : lo + 512].bitcast(F32),
                            op=mybir.AluOpType.subtract,
                        )
                    yield emit_k

                def emit_q():
                    tp = pattn.tile([P, 512], F32, tag="st", name=f"qtp_{p}")
                    qtp = tp[0:D, :]
                    for k4 in range(4):
                        nc.tensor.transpose(
                            qtp[:, P * k4 : P * (k4 + 1)], qn_tiles[(p, k4)],
                            ident,
                        )
                    lo = QW * p
                    nc.vector.tensor_copy(qtm[:, lo : lo + QW], qtp)
                    nc.vector.tensor_copy(
                        qts[D : 2 * D, lo : lo + QW], qtm[:, lo : lo + QW]
                    )
                    nc.vector.tensor_tensor(
                        out=qts[0:D, lo : lo + QW],
                        in0=qtp, in1=qtm[:, lo : lo + QW].bitcast(F32),
                        op=mybir.AluOpType.subtract,
                    )
                yield emit_q

            def interleave(tp_piece, proj_piece):
                """Emit transposes of piece tp_piece interleaved with the
                projection chains of piece proj_piece (hides evac latency:
                same-buffer reuses are always >=2 PE units apart)."""
                units = []
                tps = list(transp_units(tp_piece)) if tp_piece is not None else []
                prs = list(proj_units(proj_piece)) if proj_piece is not None else []
                # pattern: T P P T P P T P P P P  (or whatever is available)
                while tps or prs:
                    if tps:
                        units.append(tps.pop(0))
                    for _ in range(2):
                        if prs:
                            units.append(prs.pop(0))
                for u in units:
                    u()

            def attn_tail(p, t, st, out_ps, n_st):
                pt = ppool.tile([P, QW], F32, tag="pt", name=f"pt_{p}_{t}")
                if 8 * p <= t < 8 * p + 4:
                    # key-buffer slot 2p: valid or dead via exp bias 0/-1e30
                    nc.scalar.activation(
                        pt, st, mybir.ActivationFunctionType.Exp,
                        bias=flg[:, p : p + 1],
                    )
                else:
                    nc.scalar.activation(pt, st, mybir.ActivationFunctionType.Exp)
                if t >= 8 * p + 4:
                    # diagonal block: zero entries above the causal line
                    k = t - (8 * p + 4)
                    nc.vector.tensor_tensor(
                        out=pt, in0=pt, in1=dmask[:, k, :],
                        op=mybir.AluOpType.mult,
                    )
                for qb in range(NCHUNK):
                    nc.tensor.matmul(
                        out_ps[:, qb, :], pt[:, P * qb : P * (qb + 1)],
                        vp[:, t, :],
                        start=(t == 0 and qb == 0),
                        stop=(t == n_st - 1 and qb == NCHUNK - 1),
                        skip_group_check=True,
                    )

            def stage_attn(p, inject=None):
                lo = QW * p
                n_st = 8 * (p + 1)
                out_ps = pout.tile([P, NCHUNK, D + 1], F32, tag="out",
                                   name=f"out_{p}")
                st_tiles = []
                for t in range(n_st):
                    st = pattn.tile([P, QW], F32, tag="st", name=f"st_{p}_{t}")
                    nc.tensor.matmul(
                        st, kts[0:D, P * t : P * (t + 1)], qtm[:, lo : lo + QW],
                        start=True, stop=False,
                    )
                    nc.tensor.matmul(
                        st, kts[:, P * t : P * (t + 1)], qts[:, lo : lo + QW],
                        start=False, stop=True,
                    )
                    st_tiles.append(st)
                    if t == LOOKAHEAD - 1 and inject is not None:
                        # next-piece PE work runs while this chunk's first
                        # exps fill the ACT pipe (and the previous chunk's
                        # tails drain)
                        inject()
                    if t >= LOOKAHEAD:
                        attn_tail(p, t - LOOKAHEAD, st_tiles[t - LOOKAHEAD],
                                  out_ps, n_st)
                for t in range(max(0, n_st - LOOKAHEAD), n_st):
                    attn_tail(p, t, st_tiles[t], out_ps, n_st)
                nc.vector.tensor_copy(
                    o_sb[:, NCHUNK * p : NCHUNK * (p + 1), :], out_ps
                )

            # software pipeline: PE stream has no dependency gaps
            stage_load(0)
            stage_load(1)
            interleave(None, 0)
            interleave(0, 1)
            stage_load(2)
            stage_attn(0, lambda: (interleave(1, 2), stage_load(3)))
            stage_attn(1, lambda: interleave(2, 3))
            stage_attn(2, lambda: interleave(3, None))
            stage_attn(3)
            nc.sync.dma_start(o[:, :, :], o_sb)
    if not nc.is_finalized():
        nc.finalize()
    return nc


_CACHED_NC = None


def kernel(**inputs):
    global _CACHED_NC
    x = np.ascontiguousarray(np.asarray(inputs["x"], dtype=np.float32))
    WQ = np.ascontiguousarray(np.asarray(inputs["WQ"], dtype=np.float32))
    WV = np.ascontiguousarray(np.asarray(inputs["WV"], dtype=np.float32))
    B = x.shape[0]

    if _CACHED_NC is None:
        _CACHED_NC = build_nc()
    nc = _CACHED_NC

    in_maps = []
    for core in range(8):
        b, h = divmod(core, 2)
        xtb = x[b].T  # [C, T]
        cols = np.concatenate(
            [np.arange(512 * j, 512 * (j + 1)) for j in KEY_ORDER[h]]
        )
        in_maps.append(
            {
                "xt": np.ascontiguousarray(xtb[:, cols]),
                "wq": WQ,
                "wv": WV,
                "flgb": np.broadcast_to(
                    np.asarray(BIAS[h], np.float32), (P, NCHUNK)
                ).copy(),
            }
        )

    trace = os.environ.get("KERNEL_TRACE", "0") == "1"
    res = run_bass_kernel_spmd(nc, in_maps, core_ids=list(range(8)), trace=trace)
    kernel._last_results = res

    out = np.empty((B, T, D), dtype=np.float32)
    for core in range(8):
        b, h = divmod(core, 2)
        ob = res.results[core]["o"]  # [128, 16, 65]
        num = ob[:, :, :D].astype(np.float64)
        den = ob[:, :, D].astype(np.float64)
        full = (num / den[:, :, None]).astype(np.float32)  # [128, 16, 64]
        for c, j in enumerate(Q_BLOCKS[h]):
            for qb in range(NCHUNK):
                q0 = 512 * j + 128 * qb
                out[b, q0 : q0 + 128] = full[:, NCHUNK * c + qb]
    return out


# revision 59
# speedup vs baseline: 1.7205x; 1.0118x over previous
"""Causal attention head (k==v source quirk) on 8 trn2 NeuronCores.

Math per batch b:
  q = x[b] @ WQ ; kv = x[b] @ WV        (k and v are the SAME projection)
  S = q @ kv^T ; causal mask ; P = softmax(S) (no sqrt(d) scale)
  out[b] = P @ kv

Sharding: core = (b, h), h in {0,1}. Balanced causal split of the 8
512-row query blocks of batch b: h=0 gets blocks [0,2,5,7], h=1 gets
[1,3,4,6]. Keys are column-permuted per core (host side) so that every
core runs the IDENTICAL program: chunk c (c=0..3) attends to the first
1024*(c+1) keys of its permuted key buffer; the diagonal (own) block
always sits at buffer slot 2c+1 and the slot 2c block is either fully
valid or fully dead, selected by a per-core additive bias (0 / -1e30)
folded into the exp activation.

Timeline-model cost of a matmul is moving-operand columns times a
per-dtype cycles/row (fp32=4, fp32r=1 when >=256 cols); contraction
depth (<=128) is free. Hence:
  - projections run "flipped": stationary = x c-tile [128c, 128t],
    moving = W columns -> 4x fewer columns, and kv lands directly in
    natural [t, d] layout (vp), no V transposes.
  - QK runs as fp32r hi/lo limbs (11-bit each) in TWO matmuls per
    score tile: main = k_hi . q_hi (K=64), cross = k_hi . q_lo +
    k_lo . q_hi STACKED into one K=128 matmul (kts rows 0-63 = hi,
    64-127 = lo; qts rows 0-63 = q_lo, 64-127 = q_hi). Near-fp32
    logits at 2 column-passes instead of fp32's 4.
  - PV runs "flipped": stationary = P [128s, 128q] quarter-tiles,
    moving = V' [128s, 65] -> 260 cols/tile instead of 512, full fp32.
  - the diagonal causal mask is a DVE multiply with one of 4
    precomputed 0/1 mask tiles (keeps the per-tile exp->mask->PV chain
    short); dead slots die via the exp bias, so GpSimd is off the
    critical path entirely.
  - score tiles use a 3-deep lookahead (QK of tiles t..t+3 issue ahead
    of the exp/PV tail of tile t) and stages are software-pipelined
    (transposes of piece p+1 + projections of piece p+2 are emitted
    before the last attention tails of piece p) so the PE never idles
    and stays at max p-state.
Score form is transposed S^T[s, q]; softmax needs no max-subtraction
(max logit ~61); the denominator rides as a ones column appended to
V'; final divide on host in fp64 during unsharding.
"""

import os
import sys

import numpy as np

sys.path.insert(0, "/opt/trn_rl_repo")

import concourse.bass as bass
import concourse.bacc as bacc
import concourse.mybir as mybir
from concourse.bass_utils import run_bass_kernel_spmd
from concourse.tile import TileContext

P = 128
T = 4096
C = 1024
D = 64
NCTILE = C // P      # 8 contraction tiles
NCHUNK = 4           # query chunks per core (512 queries each)
QW = 512             # queries per chunk
NQ = NCHUNK * QW     # 2048 queries per core
NT = T // P          # 32 key tiles
LOOKAHEAD = 3

KEY_ORDER = {0: [1, 0, 3, 2, 4, 5, 6, 7], 1: [0, 1, 2, 3, 5, 4, 7, 6]}
Q_BLOCKS = {0: [0, 2, 5, 7], 1: [1, 3, 4, 6]}
# additive exp-bias for key-buffer slot 2c in chunk c: 0 = valid, -1e30 = dead
BIAS = {0: [-1e30, -1e30, 0.0, 0.0], 1: [0.0, 0.0, -1e30, -1e30]}

F32 = mybir.dt.float32
F32R = mybir.dt.float32r


def build_nc():
    nc = bacc.Bacc("TRN2")
    xt = nc.dram_tensor("xt", [C, T], F32, kind="ExternalInput")
    wq = nc.dram_tensor("wq", [C, D], F32, kind="ExternalInput")
    wv = nc.dram_tensor("wv", [C, D], F32, kind="ExternalInput")
    flgb = nc.dram_tensor("flgb", [P, NCHUNK], F32, kind="ExternalInput")
    o = nc.dram_tensor("o", [P, NQ // P, D + 1], F32, kind="ExternalOutput")

    with TileContext(nc) as tc:
        with (
            tc.tile_pool(name="persist", bufs=1) as persist,
            tc.tile_pool(name="xpool", bufs=2) as xpool,
            tc.tile_pool(name="qnpool", bufs=8) as qnpool,
            tc.tile_pool(name="ppool", bufs=4) as ppool,
            tc.tile_pool(name="pproj", bufs=2, space="PSUM") as pproj,
            tc.tile_pool(name="pattn", bufs=5, space="PSUM") as pattn,
            tc.tile_pool(name="pout", bufs=1, space="PSUM") as pout,
        ):
            # --- constants ---
            ident = persist.tile([P, P], F32, tag="ident", name="ident")
            nc.gpsimd.memset(ident, 1.0)
            nc.gpsimd.affine_select(
                out=ident, in_=ident, pattern=[[-1, P]],
                compare_op=mybir.AluOpType.is_equal, fill=0.0,
                base=0, channel_multiplier=1,
            )
            # 4 diagonal causal masks: dmask[k][s, c] = (c >= 128k + s)
            dmask = persist.tile([P, NCHUNK, QW], F32, tag="dmask", name="dmask")
            nc.gpsimd.memset(dmask, 1.0)
            for k in range(NCHUNK):
                nc.gpsimd.affine_select(
                    out=dmask[:, k, :], in_=dmask[:, k, :], pattern=[[1, QW]],
                    compare_op=mybir.AluOpType.is_ge, fill=0.0,
                    base=-(P * k), channel_multiplier=-1,
                )
            # weight/flag DMAs ride the ACT queue so the SP queue starts on
            # the first x piece immediately
            wvq = persist.tile([P, NCTILE, 2 * D], F32, tag="wvq", name="wvq")
            nc.scalar.dma_start(
                wvq[:, :, 0:D], wv[:, :].rearrange("(j p) d -> p j d", p=P)
            )
            nc.scalar.dma_start(
                wvq[:, :, D : 2 * D], wq[:, :].rearrange("(j p) d -> p j d", p=P)
            )
            flgb0 = persist.tile([P, NCHUNK], F32, tag="flgb0", name="flgb0")
            nc.scalar.dma_start(flgb0, flgb[:, :])
            flg = persist.tile([P, NCHUNK], F32, tag="flg", name="flg")
            nc.vector.tensor_copy(flg, flgb0)  # seed DVE clock on the DMA

            # --- persistent SBUF state ---
            vp = persist.tile([P, NT, D + 1], F32, tag="vp", name="vp")
            nc.gpsimd.memset(vp[:, :, D : D + 1], 1.0)
            # stacked fp32r limb tensors (matmul operands must share a base
            # partition, so q_hi is stored twice):
            #   kts rows 0-63 = kv_hi, rows 64-127 = kv_lo
            #   qtm           = q_hi           (main matmul moving operand)
            #   qts rows 0-63 = q_lo, rows 64-127 = q_hi (pairs with kts)
            kts = persist.tile([P, T], F32R, tag="kts", name="kts")
            qtm = persist.tile([D, NQ], F32R, tag="qtm", name="qtm")
            qts = persist.tile([P, NQ], F32R, tag="qts", name="qts")
            o_sb = persist.tile([P, NQ // P, D + 1], F32, tag="o_sb", name="o_sb")

            xtps = [None] * NCHUNK
            qn_tiles = {}

            def stage_load(p):
                xtp = xpool.tile([P, NCTILE, 1024], F32, tag="xtp", name=f"xtp_{p}")
                xtps[p] = xtp
                for e8 in range(8):
                    for ch in range(2):
                        nc.sync.dma_start(
                            xtp[:, e8 : e8 + 1, 512 * ch : 512 * ch + 512],
                            xt[
                                128 * e8 : 128 * e8 + 128,
                                1024 * p + 512 * ch : 1024 * p + 512 * ch + 512,
                            ].rearrange("(j p) c -> p j c", p=P),
                        )

            def proj_units(p):
                """Yield one projection-chain emitter per 128-key tile."""
                xtp = xtps[p]
                for tt in range(8):
                    def emit(tt=tt):
                        t = 8 * p + tt
                        own = tt >= 4
                        width = 2 * D if own else D
                        vq_ps = pproj.tile([P, 2 * D], F32, tag="vq",
                                           name=f"vq_{t}")
                        for j in range(NCTILE):
                            nc.tensor.matmul(
                                vq_ps[:, 0:width],
                                xtp[:, j, P * tt : P * (tt + 1)],
                                wvq[:, j, 0:width],
                                start=(j == 0), stop=(j == NCTILE - 1),
                            )
                        # evacuate on ACT (activation-Copy): keeps the DVE
                        # queue free for the transpose limb ops
                        nc.scalar.activation(
                            vp[:, t, 0:D], vq_ps[:, 0:D],
                            mybir.ActivationFunctionType.Copy,
                        )
                        if own:
                            qn = qnpool.tile([P, D], F32, tag="qn",
                                             name=f"qn_{t}")
                            nc.scalar.activation(
                                qn, vq_ps[:, D : 2 * D],
                                mybir.ActivationFunctionType.Copy,
                            )
                            qn_tiles[(p, tt - 4)] = qn
                    yield emit

            def transp_units(p):
                """Yield 3 transpose-batch emitters (2x kv, 1x q); the PSUM
                batch tile borrows an 'st' buffer from pattn."""
                for half in range(2):
                    def emit_k(half=half):
                        tp = pattn.tile([P, 512], F32, tag="st",
                                        name=f"ktp_{p}_{half}")
                        ktp = tp[0:D, :]
                        for k4 in range(4):
                            t = 8 * p + 4 * half + k4
                            nc.tensor.transpose(
                                ktp[:, P * k4 : P * (k4 + 1)], vp[:, t, 0:D],
                                ident,
                            )
                        lo = 1024 * p + 512 * half
                        nc.vector.tensor_copy(kts[0:D, lo : lo + 512], ktp)
                        nc.vector.tensor_tensor(
                            out=kts[D : 2 * D, lo : lo + 512],
                            in0=ktp, in1=kts[0:D, lo : lo + 512].bitcast(F32),
                            op=mybir.AluOpType.subtract,
                        )
                    yield emit_k

                def emit_q():
                    tp = pattn.tile([P, 512], F32, tag="st", name=f"qtp_{p}")
                    qtp = tp[0:D, :]
                    for k4 in range(4):
                        nc.tensor.transpose(
                            qtp[:, P * k4 : P * (k4 + 1)], qn_tiles[(p, k4)],
                            ident,
                        )
                    lo = QW * p
                    nc.vector.tensor_copy(qtm[:, lo : lo + QW], qtp)
                    nc.vector.tensor_copy(
                        qts[D : 2 * D, lo : lo + QW], qtm[:, lo : lo + QW]
                    )
                    nc.vector.tensor_tensor(
                        out=qts[0:D, lo : lo + QW],
                        in0=qtp, in1=qtm[:, lo : lo + QW].bitcast(F32),
                        op=mybir.AluOpType.subtract,
                    )
                yield emit_q

            def pipeline_units(tp_piece, proj_piece):
                """Transposes of piece tp_piece interleaved with the
                projection chains of piece proj_piece: same-buffer reuses
                are always >=2 PE units apart."""
                units = []
                tps = list(transp_units(tp_piece)) if tp_piece is not None else []
                prs = list(proj_units(proj_piece)) if proj_piece is not None else []
                # pattern: T P P P T P P P T P P  (or whatever is available)
                while tps or prs:
                    if tps:
                        units.append(tps.pop(0))
                    for _ in range(3):
                        if prs:
                            units.append(prs.pop(0))
                return units

            def interleave(tp_piece, proj_piece):
                for u in pipeline_units(tp_piece, proj_piece):
                    u()

            def attn_tail(p, t, st, out_ps, n_st):
                pt = ppool.tile([P, QW], F32, tag="pt", name=f"pt_{p}_{t}")
                if 8 * p <= t < 8 * p + 4:
                    # key-buffer slot 2p: valid or dead via exp bias 0/-1e30
                    nc.scalar.activation(
                        pt, st, mybir.ActivationFunctionType.Exp,
                        bias=flg[:, p : p + 1],
                    )
                else:
                    nc.scalar.activation(pt, st, mybir.ActivationFunctionType.Exp)
                if t >= 8 * p + 4:
                    # diagonal block: zero entries above the causal line
                    k = t - (8 * p + 4)
                    nc.vector.tensor_tensor(
                        out=pt, in0=pt, in1=dmask[:, k, :],
                        op=mybir.AluOpType.mult,
                    )
                for qb in range(NCHUNK):
                    nc.tensor.matmul(
                        out_ps[:, qb, :], pt[:, P * qb : P * (qb + 1)],
                        vp[:, t, :],
                        start=(t == 0 and qb == 0),
                        stop=(t == n_st - 1 and qb == NCHUNK - 1),
                        skip_group_check=True,
                    )

            def stage_attn(p, inject=None):
                lo = QW * p
                n_st = 8 * (p + 1)
                out_ps = pout.tile([P, NCHUNK, D + 1], F32, tag="out",
                                   name=f"out_{p}")
                st_tiles = []
                units = list(inject) if inject is not None else []
                for t in range(n_st):
                    st = pattn.tile([P, QW], F32, tag="st", name=f"st_{p}_{t}")
                    nc.tensor.matmul(
                        st, kts[0:D, P * t : P * (t + 1)], qtm[:, lo : lo + QW],
                        start=True, stop=False,
                    )
                    nc.tensor.matmul(
                        st, kts[:, P * t : P * (t + 1)], qts[:, lo : lo + QW],
                        start=False, stop=True,
                    )
                    st_tiles.append(st)
                    if t >= LOOKAHEAD - 1 and units:
                        # spread next-piece PE work through the chunk: keeps
                        # the ACT pipe fed and the st-buffer rotation free
                        units.pop(0)()
                    if t >= LOOKAHEAD:
                        attn_tail(p, t - LOOKAHEAD, st_tiles[t - LOOKAHEAD],
                                  out_ps, n_st)
                for u in units:
                    u()
                for t in range(max(0, n_st - LOOKAHEAD), n_st):
                    attn_tail(p, t, st_tiles[t], out_ps, n_st)
                nc.vector.tensor_copy(
                    o_sb[:, NCHUNK * p : NCHUNK * (p + 1), :], out_ps
                )
                # ship each chunk's output as it completes (short final tail)
                nc.sync.dma_start(
                    o[:, NCHUNK * p : NCHUNK * (p + 1), :],
                    o_sb[:, NCHUNK * p : NCHUNK * (p + 1), :],
                )

            # software pipeline: PE stream has no dependency gaps
            stage_load(0)
            stage_load(1)
            interleave(None, 0)
            interleave(0, 1)
            stage_load(2)
            stage_attn(0, [lambda: stage_load(3)] + pipeline_units(1, 2))
            stage_attn(1, pipeline_units(2, 3))
            stage_attn(2, pipeline_units(3, None))
            stage_attn(3)
    if not nc.is_finalized():
        nc.finalize()
    return nc


_CACHED_NC = None


def kernel(**inputs):
    global _CACHED_NC
    x = np.ascontiguousarray(np.asarray(inputs["x"], dtype=np.float32))
    WQ = np.ascontiguousarray(np.asarray(inputs["WQ"], dtype=np.float32))
    WV = np.ascontiguousarray(np.asarray(inputs["WV"], dtype=np.float32))
    B = x.shape[0]

    if _CACHED_NC is None:
        _CACHED_NC = build_nc()
    nc = _CACHED_NC

    in_maps = []
    for core in range(8):
        b, h = divmod(core, 2)
        xtb = x[b].T  # [C, T]
        cols = np.concatenate(
            [np.arange(512 * j, 512 * (j + 1)) for j in KEY_ORDER[h]]
        )
        in_maps.append(
            {
                "xt": np.ascontiguousarray(xtb[:, cols]),
                "wq": WQ,
                "wv": WV,
                "flgb": np.broadcast_to(
                    np.asarray(BIAS[h], np.float32), (P, NCHUNK)
                ).copy(),
            }
        )

    trace = os.environ.get("KERNEL_TRACE", "0") == "1"
    res = run_bass_kernel_spmd(nc, in_maps, core_ids=list(range(8)), trace=trace)
    kernel._last_results = res

    out = np.empty((B, T, D), dtype=np.float32)
    for core in range(8):
        b, h = divmod(core, 2)
        ob = res.results[core]["o"]  # [128, 16, 65]
        num = ob[:, :, :D].astype(np.float64)
        den = ob[:, :, D].astype(np.float64)
        full = (num / den[:, :, None]).astype(np.float32)  # [128, 16, 64]
        for c, j in enumerate(Q_BLOCKS[h]):
            for qb in range(NCHUNK):
                q0 = 512 * j + 128 * qb
                out[b, q0 : q0 + 128] = full[:, NCHUNK * c + qb]
    return out


# revision 70
# speedup vs baseline: 1.7985x; 1.0454x over previous
"""Causal attention head (k==v source quirk) on 8 trn2 NeuronCores.

Math per batch b:
  q = x[b] @ WQ ; kv = x[b] @ WV        (k and v are the SAME projection)
  S = q @ kv^T ; causal mask ; P = softmax(S) (no sqrt(d) scale)
  out[b] = P @ kv

Sharding: core = (b, h), h in {0,1}. Balanced causal split of the 8
512-row query blocks of batch b: h=0 gets blocks [0,2,5,7], h=1 gets
[1,3,4,6]. Keys are column-permuted per core (host side) so that every
core runs the IDENTICAL program: chunk c (c=0..3) attends to the first
1024*(c+1) keys of its permuted key buffer; the diagonal (own) block
always sits at buffer slot 2c+1 and the slot 2c block is either fully
valid or fully dead, selected by a per-core additive bias (0 / -1e30)
folded into the exp activation.

Timeline-model cost of a matmul is moving-operand columns times a
per-dtype cycles/row (fp32=4, fp32r=1 when >=256 cols); contraction
depth (<=128) is free. Hence:
  - projections run "flipped": stationary = x c-tile [128c, 128t],
    moving = W columns -> 4x fewer columns, and kv lands directly in
    natural [t, d] layout (vp), no V transposes.
  - QK runs as fp32r hi/lo limbs (11-bit each) in TWO matmuls per
    score tile: main = k_hi . q_hi (K=64), cross = k_hi . q_lo +
    k_lo . q_hi STACKED into one K=128 matmul (kts rows 0-63 = hi,
    64-127 = lo; qts rows 0-63 = q_lo, 64-127 = q_hi). Near-fp32
    logits at 2 column-passes instead of fp32's 4.
  - PV runs "flipped": stationary = P [128s, 128q] quarter-tiles,
    moving = V' [128s, 65] -> 260 cols/tile instead of 512, full fp32.
  - the diagonal causal mask is a DVE multiply with one of 4
    precomputed 0/1 mask tiles (keeps the per-tile exp->mask->PV chain
    short); dead slots die via the exp bias, so GpSimd is off the
    critical path entirely.
  - score tiles use a 3-deep lookahead (QK of tiles t..t+3 issue ahead
    of the exp/PV tail of tile t) and stages are software-pipelined
    (transposes of piece p+1 + projections of piece p+2 are emitted
    before the last attention tails of piece p) so the PE never idles
    and stays at max p-state.
Score form is transposed S^T[s, q]; softmax needs no max-subtraction
(max logit ~61); the denominator rides as a ones column appended to
V'; final divide on host in fp64 during unsharding.
"""

import os
import sys

import numpy as np

sys.path.insert(0, "/opt/trn_rl_repo")

import concourse.bass as bass
import concourse.bacc as bacc
import concourse.mybir as mybir
from concourse.bass_utils import run_bass_kernel_spmd
from concourse.tile import TileContext

P = 128
T = 4096
C = 1024
D = 64
NCTILE = C // P      # 8 contraction tiles
NCHUNK = 4           # query chunks per core (512 queries each)
QW = 512             # queries per chunk
NQ = NCHUNK * QW     # 2048 queries per core
NT = T // P          # 32 key tiles
LOOKAHEAD = 3

KEY_ORDER = {0: [1, 0, 3, 2, 4, 5, 6, 7], 1: [0, 1, 2, 3, 5, 4, 7, 6]}
Q_BLOCKS = {0: [0, 2, 5, 7], 1: [1, 3, 4, 6]}
# additive exp-bias for key-buffer slot 2c in chunk c: 0 = valid, -1e30 = dead
BIAS = {0: [-1e30, -1e30, 0.0, 0.0], 1: [0.0, 0.0, -1e30, -1e30]}

F32 = mybir.dt.float32
F32R = mybir.dt.float32r


def build_nc():
    nc = bacc.Bacc("TRN2")
    xt = nc.dram_tensor("xt", [C, T], F32, kind="ExternalInput")
    wq = nc.dram_tensor("wq", [C, D], F32, kind="ExternalInput")
    wv = nc.dram_tensor("wv", [C, D], F32, kind="ExternalInput")
    flgb = nc.dram_tensor("flgb", [P, NCHUNK], F32, kind="ExternalInput")
    o = nc.dram_tensor("o", [P, NQ // P, D + 1], F32, kind="ExternalOutput")

    with TileContext(nc) as tc:
        with (
            tc.tile_pool(name="persist", bufs=1) as persist,
            tc.tile_pool(name="xpool", bufs=2) as xpool,
            tc.tile_pool(name="qnpool", bufs=8) as qnpool,
            tc.tile_pool(name="ppool", bufs=4) as ppool,
            tc.tile_pool(name="pproj", bufs=2, space="PSUM") as pproj,
            tc.tile_pool(name="pattn", bufs=5, space="PSUM") as pattn,
            tc.tile_pool(name="pout", bufs=1, space="PSUM") as pout,
        ):
            # --- constants ---
            # ones column of V' first: the first PV tail waits on it
            vp = persist.tile([P, NT, D + 1], F32, tag="vp", name="vp")
            nc.gpsimd.memset(vp[:, :, D : D + 1], 1.0)
            ident = persist.tile([P, P], F32, tag="ident", name="ident")
            nc.gpsimd.memset(ident, 1.0)
            nc.gpsimd.affine_select(
                out=ident, in_=ident, pattern=[[-1, P]],
                compare_op=mybir.AluOpType.is_equal, fill=0.0,
                base=0, channel_multiplier=1,
            )
            # 4 diagonal causal masks: dmask[k][s, c] = (c >= 128k + s)
            dmask = persist.tile([P, NCHUNK, QW], F32, tag="dmask", name="dmask")
            nc.gpsimd.memset(dmask, 1.0)
            for k in range(NCHUNK):
                nc.gpsimd.affine_select(
                    out=dmask[:, k, :], in_=dmask[:, k, :], pattern=[[1, QW]],
                    compare_op=mybir.AluOpType.is_ge, fill=0.0,
                    base=-(P * k), channel_multiplier=-1,
                )
            # weight/flag DMAs ride the ACT queue so the SP queue starts on
            # the first x piece immediately
            wvq = persist.tile([P, NCTILE, 2 * D], F32, tag="wvq", name="wvq")
            nc.scalar.dma_start(
                wvq[:, :, 0:D], wv[:, :].rearrange("(j p) d -> p j d", p=P)
            )
            nc.scalar.dma_start(
                wvq[:, :, D : 2 * D], wq[:, :].rearrange("(j p) d -> p j d", p=P)
            )
            flgb0 = persist.tile([P, NCHUNK], F32, tag="flgb0", name="flgb0")
            nc.scalar.dma_start(flgb0, flgb[:, :])
            flg = persist.tile([P, NCHUNK], F32, tag="flg", name="flg")
            nc.vector.tensor_copy(flg, flgb0)  # seed DVE clock on the DMA

            # --- persistent SBUF state ---
            # stacked fp32r limb tensors (matmul operands must share a base
            # partition, so q_hi is stored twice):
            #   kts rows 0-63 = kv_hi, rows 64-127 = kv_lo
            #   qtm           = q_hi           (main matmul moving operand)
            #   qts rows 0-63 = q_lo, rows 64-127 = q_hi (pairs with kts)
            kts = persist.tile([P, T], F32R, tag="kts", name="kts")
            qtm = persist.tile([D, NQ], F32R, tag="qtm", name="qtm")
            qts = persist.tile([P, NQ], F32R, tag="qts", name="qts")
            o_sb = persist.tile([P, NQ // P, D + 1], F32, tag="o_sb", name="o_sb")

            xtps = [None] * NCHUNK
            qn_tiles = {}

            def stage_load(p):
                xtp = xpool.tile([P, NCTILE, 1024], F32, tag="xtp", name=f"xtp_{p}")
                xtps[p] = xtp
                if p == 0:
                    # piece 0 in 128-col t-slices across all c-tiles: chain tt
                    # only waits on DMA tt (512B descriptors, same bandwidth)
                    for k in range(8):
                        nc.sync.dma_start(
                            xtp[:, :, 128 * k : 128 * (k + 1)],
                            xt[:, 128 * k : 128 * (k + 1)].rearrange(
                                "(j p) c -> p j c", p=P
                            ),
                        )
                else:
                    # ch-major: the first 8 DMAs cover cols 0-512 of every
                    # c-tile
                    for ch in range(2):
                        for e8 in range(8):
                            nc.sync.dma_start(
                                xtp[:, e8 : e8 + 1, 512 * ch : 512 * ch + 512],
                                xt[
                                    128 * e8 : 128 * e8 + 128,
                                    1024 * p + 512 * ch : 1024 * p + 512 * ch + 512,
                                ].rearrange("(j p) c -> p j c", p=P),
                            )

            def proj_units(p):
                """Yield one projection-chain emitter per 128-key tile."""
                xtp = xtps[p]
                for tt in range(8):
                    def emit(tt=tt):
                        t = 8 * p + tt
                        own = tt >= 4
                        width = 2 * D if own else D
                        vq_ps = pproj.tile([P, 2 * D], F32, tag="vq",
                                           name=f"vq_{t}")
                        for j in range(NCTILE):
                            nc.tensor.matmul(
                                vq_ps[:, 0:width],
                                xtp[:, j, P * tt : P * (tt + 1)],
                                wvq[:, j, 0:width],
                                start=(j == 0), stop=(j == NCTILE - 1),
                            )
                        # evacuate on ACT (activation-Copy): keeps the DVE
                        # queue free for the transpose limb ops
                        nc.scalar.activation(
                            vp[:, t, 0:D], vq_ps[:, 0:D],
                            mybir.ActivationFunctionType.Copy,
                        )
                        if own:
                            qn = qnpool.tile([P, D], F32, tag="qn",
                                             name=f"qn_{t}")
                            nc.scalar.activation(
                                qn, vq_ps[:, D : 2 * D],
                                mybir.ActivationFunctionType.Copy,
                            )
                            qn_tiles[(p, tt - 4)] = qn
                    yield emit

            def transp_units(p):
                """Yield 3 transpose-batch emitters (2x kv, 1x q); the PSUM
                batch tile borrows an 'st' buffer from pattn."""
                for half in range(2):
                    def emit_k(half=half):
                        tp = pattn.tile([P, 512], F32, tag="st",
                                        name=f"ktp_{p}_{half}")
                        ktp = tp[0:D, :]
                        for k4 in range(4):
                            t = 8 * p + 4 * half + k4
                            nc.tensor.transpose(
                                ktp[:, P * k4 : P * (k4 + 1)], vp[:, t, 0:D],
                                ident,
                            )
                        lo = 1024 * p + 512 * half
                        nc.vector.tensor_copy(kts[0:D, lo : lo + 512], ktp)
                        nc.vector.tensor_tensor(
                            out=kts[D : 2 * D, lo : lo + 512],
                            in0=ktp, in1=kts[0:D, lo : lo + 512].bitcast(F32),
                            op=mybir.AluOpType.subtract,
                        )
                    yield emit_k

                def emit_q():
                    tp = pattn.tile([P, 512], F32, tag="st", name=f"qtp_{p}")
                    qtp = tp[0:D, :]
                    for k4 in range(4):
                        nc.tensor.transpose(
                            qtp[:, P * k4 : P * (k4 + 1)], qn_tiles[(p, k4)],
                            ident,
                        )
                    lo = QW * p
                    nc.vector.tensor_copy(qtm[:, lo : lo + QW], qtp)
                    nc.vector.tensor_copy(
                        qts[D : 2 * D, lo : lo + QW], qtm[:, lo : lo + QW]
                    )
                    nc.vector.tensor_tensor(
                        out=qts[0:D, lo : lo + QW],
                        in0=qtp, in1=qtm[:, lo : lo + QW].bitcast(F32),
                        op=mybir.AluOpType.subtract,
                    )
                yield emit_q

            def pipeline_units(tp_piece, proj_piece):
                """Transposes of piece tp_piece interleaved with the
                projection chains of piece proj_piece: same-buffer reuses
                are always >=2 PE units apart."""
                units = []
                tps = list(transp_units(tp_piece)) if tp_piece is not None else []
                prs = list(proj_units(proj_piece)) if proj_piece is not None else []
                # pattern: T P T P T P P P P P P — transposes early (next
                # chunk's kts limbs ready sooner; proj units late, after
                # their DMA piece lands), one proj between T's to hide the
                # transpose-batch evac latency
                while tps:
                    units.append(tps.pop(0))
                    if prs:
                        units.append(prs.pop(0))
                units.extend(prs)
                return units

            def interleave(tp_piece, proj_piece):
                for u in pipeline_units(tp_piece, proj_piece):
                    u()

            def attn_tail(p, t, st, out_ps, first, last):
                pt = ppool.tile([P, QW], F32, tag="pt", name=f"pt_{p}_{t}")
                if 8 * p <= t < 8 * p + 4:
                    # key-buffer slot 2p: valid or dead via exp bias 0/-1e30
                    nc.scalar.activation(
                        pt, st, mybir.ActivationFunctionType.Exp,
                        bias=flg[:, p : p + 1],
                    )
                else:
                    nc.scalar.activation(pt, st, mybir.ActivationFunctionType.Exp)
                if t >= 8 * p + 4:
                    # diagonal block: zero entries above the causal line
                    k = t - (8 * p + 4)
                    nc.vector.tensor_tensor(
                        out=pt, in0=pt, in1=dmask[:, k, :],
                        op=mybir.AluOpType.mult,
                    )
                for qb in range(NCHUNK):
                    nc.tensor.matmul(
                        out_ps[:, qb, :], pt[:, P * qb : P * (qb + 1)],
                        vp[:, t, :],
                        start=(first and qb == 0),
                        stop=(last and qb == NCHUNK - 1),
                        skip_group_check=True,
                    )

            def stage_attn(p, inject=None):
                lo = QW * p
                n_st = 8 * (p + 1)
                out_ps = pout.tile([P, NCHUNK, D + 1], F32, tag="out",
                                   name=f"out_{p}")
                # masked tiles (dead slot 2p + diagonal slot 2p+1) first:
                # their longer exp->mask->PV chains overlap the injected
                # pipeline work, and the chunk drains on cheap maskless tails
                order = list(range(8 * p, n_st)) + list(range(0, 8 * p))
                sts = {}
                units = list(inject) if inject is not None else []
                for i, t in enumerate(order):
                    st = pattn.tile([P, QW], F32, tag="st", name=f"st_{p}_{t}")
                    nc.tensor.matmul(
                        st, kts[0:D, P * t : P * (t + 1)], qtm[:, lo : lo + QW],
                        start=True, stop=False,
                    )
                    nc.tensor.matmul(
                        st, kts[:, P * t : P * (t + 1)], qts[:, lo : lo + QW],
                        start=False, stop=True,
                    )
                    sts[t] = st
                    if i >= LOOKAHEAD - 1 and units:
                        # spread next-piece PE work through the chunk: keeps
                        # the ACT pipe fed and the st-buffer rotation free
                        units.pop(0)()
                    if i >= LOOKAHEAD:
                        tl = order[i - LOOKAHEAD]
                        attn_tail(p, tl, sts.pop(tl), out_ps,
                                  first=(i == LOOKAHEAD), last=False)
                for u in units:
                    u()
                drain = order[max(0, len(order) - LOOKAHEAD):]
                for i, t in enumerate(drain):
                    attn_tail(p, t, sts.pop(t), out_ps,
                              first=(len(order) <= LOOKAHEAD and i == 0),
                              last=(i == len(drain) - 1))
                nc.vector.tensor_copy(
                    o_sb[:, NCHUNK * p : NCHUNK * (p + 1), :], out_ps
                )
                # ship each chunk's output as it completes (short final tail)
                nc.sync.dma_start(
                    o[:, NCHUNK * p : NCHUNK * (p + 1), :],
                    o_sb[:, NCHUNK * p : NCHUNK * (p + 1), :],
                )

            # software pipeline: PE stream has no dependency gaps
            stage_load(0)
            stage_load(1)
            interleave(None, 0)
            interleave(0, 1)
            stage_load(2)
            stage_attn(0, [lambda: stage_load(3)] + pipeline_units(1, 2))
            stage_attn(1, pipeline_units(2, 3))
            stage_attn(2, pipeline_units(3, None))
            stage_attn(3)
    if not nc.is_finalized():
        nc.finalize()
    return nc


_CACHED_NC = None


def kernel(**inputs):
    global _CACHED_NC
    x = np.ascontiguousarray(np.asarray(inputs["x"], dtype=np.float32))
    WQ = np.ascontiguousarray(np.asarray(inputs["WQ"], dtype=np.float32))
    WV = np.ascontiguousarray(np.asarray(inputs["WV"], dtype=np.float32))
    B = x.shape[0]

    if _CACHED_NC is None:
        _CACHED_NC = build_nc()
    nc = _CACHED_NC

    in_maps = []
    for core in range(8):
        b, h = divmod(core, 2)
        xtb = x[b].T  # [C, T]
        cols = np.concatenate(
            [np.arange(512 * j, 512 * (j + 1)) for j in KEY_ORDER[h]]
        )
        in_maps.append(
            {
                "xt": np.ascontiguousarray(xtb[:, cols]),
                "wq": WQ,
                "wv": WV,
                "flgb": np.broadcast_to(
                    np.asarray(BIAS[h], np.float32), (P, NCHUNK)
                ).copy(),
            }
        )

    trace = os.environ.get("KERNEL_TRACE", "0") == "1"
    res = run_bass_kernel_spmd(nc, in_maps, core_ids=list(range(8)), trace=trace)
    kernel._last_results = res

    out = np.empty((B, T, D), dtype=np.float32)
    for core in range(8):
        b, h = divmod(core, 2)
        ob = res.results[core]["o"]  # [128, 16, 65]
        num = ob[:, :, :D].astype(np.float64)
        den = ob[:, :, D].astype(np.float64)
        full = (num / den[:, :, None]).astype(np.float32)  # [128, 16, 64]
        for c, j in enumerate(Q_BLOCKS[h]):
            for qb in range(NCHUNK):
                q0 = 512 * j + 128 * qb
                out[b, q0 : q0 + 128] = full[:, NCHUNK * c + qb]
    return out


# revision 77
# speedup vs baseline: 1.9173x; 1.0660x over previous
"""Causal attention head (k==v source quirk) on 8 trn2 NeuronCores.

Math per batch b:
  q = x[b] @ WQ ; kv = x[b] @ WV        (k and v are the SAME projection)
  S = q @ kv^T ; causal mask ; P = softmax(S) (no sqrt(d) scale)
  out[b] = P @ kv

Sharding: core = (b, h), h in {0,1}. Balanced causal split of the 8
512-row query blocks of batch b: h=0 gets blocks [0,2,5,7], h=1 gets
[1,3,4,6]. Keys are column-permuted per core (host side) so that every
core runs the IDENTICAL program: chunk c (c=0..3) attends to the first
1024*(c+1) keys of its permuted key buffer; the diagonal (own) block
always sits at buffer slot 2c+1 and the slot 2c block is either fully
valid or fully dead, selected by a per-core additive bias (0 / -1e30)
folded into the exp activation.

Timeline-model cost of a matmul is moving-operand columns times a
per-dtype cycles/row (fp32=4, fp32r=1 when >=256 cols); contraction
depth (<=128) is free. Hence:
  - projections run "flipped": stationary = x c-tile [128c, 128t],
    moving = W columns -> 4x fewer columns, and kv lands directly in
    natural [t, d] layout (vp), no V transposes.
  - QK runs as fp32r hi/lo limbs (11-bit each) in TWO matmuls per
    score tile: main = k_hi . q_hi (K=64), cross = k_hi . q_lo +
    k_lo . q_hi STACKED into one K=128 matmul (kts rows 0-63 = hi,
    64-127 = lo; qts rows 0-63 = q_lo, 64-127 = q_hi). Near-fp32
    logits at 2 column-passes instead of fp32's 4.
  - PV runs "flipped": stationary = P [128s, 128q] quarter-tiles,
    moving = V' [128s, 65] -> 260 cols/tile instead of 512, full fp32.
  - the diagonal causal mask is a DVE multiply with one of 4
    precomputed 0/1 mask tiles (keeps the per-tile exp->mask->PV chain
    short); dead slots die via the exp bias, so GpSimd only runs setup
    memsets/affine_selects.
  - engines are specialized so no queue blocks another: ACT = exps +
    projection-chain evacuations (activation-Copy), DVE = diagonal
    masks + fp32r limb splits, Pool = constant setup, SP = x DMAs
    (weights ride the ACT DMA queue).
  - score tiles use a 3-deep lookahead (QK of tiles t..t+3 issue ahead
    of the exp/PV tail of tile t); within each chunk the masked tiles
    (slots 2c, 2c+1) are processed FIRST so their longer tails overlap
    injected pipeline work and the chunk drains on maskless tails; the
    transposes of piece p+1 and projections of piece p+2 are spread
    between the QK tiles of chunk p (transposes first) so the PE never
    idles and stays at max p-state. Projection chains alternate two
    PSUM banks (sharing one bank serializes on the zero-region).
  - piece 0 is DMA'd in 128-column t-slices so projection chain tt
    waits on exactly one DMA; later pieces load ch-major, prefetched
    two chunks ahead.
Score form is transposed S^T[s, q]; softmax needs no max-subtraction
(max logit ~61); the denominator rides as a ones column appended to
V'; final divide on host in fp64 during unsharding.
"""

import os
import sys

import numpy as np

sys.path.insert(0, "/opt/trn_rl_repo")

import concourse.bass as bass
import concourse.bacc as bacc
import concourse.mybir as mybir
from concourse.bass_utils import run_bass_kernel_spmd
from concourse.tile import TileContext

P = 128
T = 4096
C = 1024
D = 64
NCTILE = C // P      # 8 contraction tiles
NCHUNK = 4           # query chunks per core (512 queries each)
QW = 512             # queries per chunk
NQ = NCHUNK * QW     # 2048 queries per core
NT = T // P          # 32 key tiles
LOOKAHEAD = 3

KEY_ORDER = {0: [1, 0, 3, 2, 4, 5, 6, 7], 1: [0, 1, 2, 3, 5, 4, 7, 6]}
Q_BLOCKS = {0: [0, 2, 5, 7], 1: [1, 3, 4, 6]}
# additive exp-bias for key-buffer slot 2c in chunk c: 0 = valid, -1e30 = dead
BIAS = {0: [-1e30, -1e30, 0.0, 0.0], 1: [0.0, 0.0, -1e30, -1e30]}

F32 = mybir.dt.float32
F32R = mybir.dt.float32r
F16 = mybir.dt.float16


def build_nc():
    nc = bacc.Bacc("TRN2")
    # x and the stacked [WV|WQ] arrive as host-split fp16 hi/lo limb pairs:
    # 11-bit limbs -> 21-bit effective operands (r3-grade precision) while
    # fp16 matmuls run 1 cycle/row at any moving width (fp32 needs 4).
    # Same total DMA bytes and SBUF as single fp32 tensors.
    xth = nc.dram_tensor("xth", [C, T], F16, kind="ExternalInput")
    xtl = nc.dram_tensor("xtl", [C, T], F16, kind="ExternalInput")
    whl = nc.dram_tensor("whl", [C, 2, 2 * D], F16, kind="ExternalInput")
    flgb = nc.dram_tensor("flgb", [P, NCHUNK], F32, kind="ExternalInput")
    o = nc.dram_tensor("o", [P, NQ // P, D + 1], F32, kind="ExternalOutput")

    with TileContext(nc) as tc:
        with (
            tc.tile_pool(name="persist", bufs=1) as persist,
            tc.tile_pool(name="xpool", bufs=2) as xpool,
            tc.tile_pool(name="qnpool", bufs=8) as qnpool,
            tc.tile_pool(name="spool", bufs=2) as spool,
            tc.tile_pool(name="ppool", bufs=4) as ppool,
            tc.tile_pool(name="pproj", bufs=2, space="PSUM") as pproj,
            tc.tile_pool(name="pattn", bufs=5, space="PSUM") as pattn,
            tc.tile_pool(name="pout", bufs=1, space="PSUM") as pout,
        ):
            # --- constants ---
            # ones column of V' first: the first PV tail waits on it
            vp = persist.tile([P, NT, D + 1], F32, tag="vp", name="vp")
            nc.gpsimd.memset(vp[:, :, D : D + 1], 1.0)
            ident = persist.tile([P, P], F32, tag="ident", name="ident")
            nc.gpsimd.memset(ident, 1.0)
            nc.gpsimd.affine_select(
                out=ident, in_=ident, pattern=[[-1, P]],
                compare_op=mybir.AluOpType.is_equal, fill=0.0,
                base=0, channel_multiplier=1,
            )
            # 4 diagonal causal masks: dmask[k][s, c] = (c >= 128k + s)
            dmask = persist.tile([P, NCHUNK, QW], F32, tag="dmask", name="dmask")
            nc.gpsimd.memset(dmask, 1.0)
            for k in range(NCHUNK):
                nc.gpsimd.affine_select(
                    out=dmask[:, k, :], in_=dmask[:, k, :], pattern=[[1, QW]],
                    compare_op=mybir.AluOpType.is_ge, fill=0.0,
                    base=-(P * k), channel_multiplier=-1,
                )
            # weight/flag DMAs ride the ACT queue so the SP queue starts on
            # the first x piece immediately
            wvq = persist.tile([P, NCTILE, 2, 2 * D], F16, tag="wvq",
                               name="wvq")
            nc.scalar.dma_start(
                wvq, whl[:, :, :].rearrange("(j p) l d -> p j l d", p=P)
            )
            flgb0 = persist.tile([P, NCHUNK], F32, tag="flgb0", name="flgb0")
            nc.scalar.dma_start(flgb0, flgb[:, :])
            flg = persist.tile([P, NCHUNK], F32, tag="flg", name="flg")
            nc.vector.tensor_copy(flg, flgb0)  # seed DVE clock on the DMA

            # --- persistent SBUF state ---
            # stacked fp32r limb tensors (matmul operands must share a base
            # partition, so q_hi is stored twice):
            #   kts rows 0-63 = kv_hi, rows 64-127 = kv_lo
            #   qtm           = q_hi           (main matmul moving operand)
            #   qts rows 0-63 = q_lo, rows 64-127 = q_hi (pairs with kts)
            kts = persist.tile([P, T], F32R, tag="kts", name="kts")
            qtm = persist.tile([D, NQ], F32R, tag="qtm", name="qtm")
            qts = persist.tile([P, NQ], F32R, tag="qts", name="qts")
            o_sb = persist.tile([P, NQ // P, D + 1], F32, tag="o_sb", name="o_sb")

            xtps = [None] * NCHUNK
            qn_tiles = {}

            def stage_load(p):
                xtph = xpool.tile([P, NCTILE, 1024], F16, tag="xtph",
                                  name=f"xtph_{p}")
                xtpl = xpool.tile([P, NCTILE, 1024], F16, tag="xtpl",
                                  name=f"xtpl_{p}")
                xtps[p] = (xtph, xtpl)
                if p <= 1:
                    # startup pieces in 256-col t-slices across all c-tiles:
                    # pair-chain pr waits only on its own hi/lo slice pair
                    for k in range(4):
                        for xtp, xsrc in ((xtph, xth), (xtpl, xtl)):
                            nc.sync.dma_start(
                                xtp[:, :, 256 * k : 256 * (k + 1)],
                                xsrc[
                                    :, 1024 * p + 256 * k : 1024 * p + 256 * (k + 1)
                                ].rearrange("(j p) c -> p j c", p=P),
                            )
                else:
                    # deep-prefetched pieces: coarse per-c-tile DMAs
                    for e8 in range(8):
                        for xtp, xsrc in ((xtph, xth), (xtpl, xtl)):
                            nc.sync.dma_start(
                                xtp[:, e8 : e8 + 1, :],
                                xsrc[
                                    128 * e8 : 128 * e8 + 128,
                                    1024 * p : 1024 * (p + 1),
                                ].rearrange("(j p) c -> p j c", p=P),
                            )

            def proj_units(p):
                """Yield one projection emitter per PAIR of 128-key tiles.
                fp16 limb scheme, 2 matmuls per (tile, c-tile): pass A
                streams [wh|wl] into two PSUM column groups (g0 = xh.wh,
                g1 = xh.wl), pass B accumulates xl.wh onto g0; the
                evacuation sums the groups. Two tiles share one PSUM bank
                (start only on the first write) so chains are long enough
                to hide the evac round-trip in the 2-bank rotation."""
                xtph, xtpl = xtps[p]
                for pr in range(4):
                    def emit(pr=pr):
                        own = pr >= 2
                        width = 2 * D if own else D
                        t0 = 8 * p + 2 * pr
                        vq_ps = pproj.tile([P, 2, 2, 2 * D], F32, tag="vq",
                                           name=f"vq_{t0}")
                        for half in range(2):
                            sl = slice(P * (2 * pr + half),
                                       P * (2 * pr + half + 1))
                            for j in range(NCTILE):
                                nc.tensor.matmul(
                                    vq_ps[:, half, :, 0:width],
                                    xtph[:, j, sl], wvq[:, j, :, 0:width],
                                    start=(half == 0 and j == 0), stop=False,
                                    skip_group_check=True,
                                )
                                nc.tensor.matmul(
                                    vq_ps[:, half, 0, 0:width],
                                    xtpl[:, j, sl], wvq[:, j, 0, 0:width],
                                    start=False,
                                    stop=(half == 1 and j == NCTILE - 1),
                                    skip_group_check=True,
                                )
                        # DVE may read only one PSUM operand: stage group 1
                        # through SBUF via an ACT copy, then add on DVE
                        sc = spool.tile([P, 2, 2 * D], F32, tag="sc",
                                        name=f"sc_{t0}")
                        nc.scalar.activation(
                            sc[:, :, 0:width], vq_ps[:, :, 1, 0:width],
                            mybir.ActivationFunctionType.Copy,
                        )
                        nc.vector.tensor_tensor(
                            out=vp[:, t0 : t0 + 2, 0:D],
                            in0=vq_ps[:, :, 0, 0:D], in1=sc[:, :, 0:D],
                            op=mybir.AluOpType.add,
                        )
                        if own:
                            qn = qnpool.tile([P, 2, D], F32, tag="qn",
                                             name=f"qn_{t0}")
                            nc.vector.tensor_tensor(
                                out=qn, in0=vq_ps[:, :, 0, D : 2 * D],
                                in1=sc[:, :, D : 2 * D],
                                op=mybir.AluOpType.add,
                            )
                            qn_tiles[(p, pr - 2)] = qn
                    yield emit

            def transp_units(p):
                """Yield 3 transpose-batch emitters (2x kv, 1x q); the PSUM
                batch tile borrows an 'st' buffer from pattn."""
                for half in range(2):
                    def emit_k(half=half):
                        tp = pattn.tile([P, 512], F32, tag="st",
                                        name=f"ktp_{p}_{half}")
                        ktp = tp[0:D, :]
                        for k4 in range(4):
                            t = 8 * p + 4 * half + k4
                            nc.tensor.transpose(
                                ktp[:, P * k4 : P * (k4 + 1)], vp[:, t, 0:D],
                                ident,
                            )
                        lo = 1024 * p + 512 * half
                        nc.vector.tensor_copy(kts[0:D, lo : lo + 512], ktp)
                        nc.vector.tensor_tensor(
                            out=kts[D : 2 * D, lo : lo + 512],
                            in0=ktp, in1=kts[0:D, lo : lo + 512].bitcast(F32),
                            op=mybir.AluOpType.subtract,
                        )
                    yield emit_k

                def emit_q():
                    tp = pattn.tile([P, 512], F32, tag="st", name=f"qtp_{p}")
                    qtp = tp[0:D, :]
                    for k4 in range(4):
                        nc.tensor.transpose(
                            qtp[:, P * k4 : P * (k4 + 1)],
                            qn_tiles[(p, k4 // 2)][:, k4 % 2, :],
                            ident,
                        )
                    lo = QW * p
                    nc.vector.tensor_copy(qtm[:, lo : lo + QW], qtp)
                    nc.vector.tensor_copy(
                        qts[D : 2 * D, lo : lo + QW], qtm[:, lo : lo + QW]
                    )
                    nc.vector.tensor_tensor(
                        out=qts[0:D, lo : lo + QW],
                        in0=qtp, in1=qtm[:, lo : lo + QW].bitcast(F32),
                        op=mybir.AluOpType.subtract,
                    )
                yield emit_q

            def pipeline_units(tp_piece, proj_piece):
                """Transposes of piece tp_piece interleaved with the
                projection chains of piece proj_piece: same-buffer reuses
                are always >=2 PE units apart."""
                units = []
                tps = list(transp_units(tp_piece)) if tp_piece is not None else []
                prs = list(proj_units(proj_piece)) if proj_piece is not None else []
                # pattern: T P T P T P P P P P P — transposes early (next
                # chunk's kts limbs ready sooner; proj units late, after
                # their DMA piece lands), one proj between T's to hide the
                # transpose-batch evac latency
                while tps:
                    units.append(tps.pop(0))
                    if prs:
                        units.append(prs.pop(0))
                units.extend(prs)
                return units

            def interleave(tp_piece, proj_piece):
                for u in pipeline_units(tp_piece, proj_piece):
                    u()

            def attn_tail(p, t, st, out_ps, first, last):
                pt = ppool.tile([P, QW], F32, tag="pt", name=f"pt_{p}_{t}")
                if 8 * p <= t < 8 * p + 4:
                    # key-buffer slot 2p: valid or dead via exp bias 0/-1e30
                    nc.scalar.activation(
                        pt, st, mybir.ActivationFunctionType.Exp,
                        bias=flg[:, p : p + 1],
                    )
                else:
                    nc.scalar.activation(pt, st, mybir.ActivationFunctionType.Exp)
                if t >= 8 * p + 4:
                    # diagonal block: zero entries above the causal line
                    k = t - (8 * p + 4)
                    nc.vector.tensor_tensor(
                        out=pt, in0=pt, in1=dmask[:, k, :],
                        op=mybir.AluOpType.mult,
                    )
                for qb in range(NCHUNK):
                    nc.tensor.matmul(
                        out_ps[:, qb, :], pt[:, P * qb : P * (qb + 1)],
                        vp[:, t, :],
                        start=(first and qb == 0),
                        stop=(last and qb == NCHUNK - 1),
                        skip_group_check=True,
                    )

            def stage_attn(p, inject=None):
                lo = QW * p
                n_st = 8 * (p + 1)
                out_ps = pout.tile([P, NCHUNK, D + 1], F32, tag="out",
                                   name=f"out_{p}")
                # masked tiles (dead slot 2p + diagonal slot 2p+1) first:
                # their longer exp->mask->PV chains overlap the injected
                # pipeline work, and the chunk drains on cheap maskless tails
                order = list(range(8 * p, n_st)) + list(range(0, 8 * p))
                sts = {}
                units = list(inject) if inject is not None else []
                for i, t in enumerate(order):
                    st = pattn.tile([P, QW], F32, tag="st", name=f"st_{p}_{t}")
                    nc.tensor.matmul(
                        st, kts[0:D, P * t : P * (t + 1)], qtm[:, lo : lo + QW],
                        start=True, stop=False,
                    )
                    nc.tensor.matmul(
                        st, kts[:, P * t : P * (t + 1)], qts[:, lo : lo + QW],
                        start=False, stop=True,
                    )
                    sts[t] = st
                    if i >= LOOKAHEAD - 1 and units:
                        # spread next-piece PE work through the chunk: keeps
                        # the ACT pipe fed and the st-buffer rotation free
                        units.pop(0)()
                    if i >= LOOKAHEAD:
                        tl = order[i - LOOKAHEAD]
                        attn_tail(p, tl, sts.pop(tl), out_ps,
                                  first=(i == LOOKAHEAD), last=False)
                for u in units:
                    u()
                drain = order[max(0, len(order) - LOOKAHEAD):]
                for i, t in enumerate(drain):
                    attn_tail(p, t, sts.pop(t), out_ps,
                              first=(len(order) <= LOOKAHEAD and i == 0),
                              last=(i == len(drain) - 1))
                nc.vector.tensor_copy(
                    o_sb[:, NCHUNK * p : NCHUNK * (p + 1), :], out_ps
                )
                # ship each chunk's output as it completes (short final tail)
                nc.sync.dma_start(
                    o[:, NCHUNK * p : NCHUNK * (p + 1), :],
                    o_sb[:, NCHUNK * p : NCHUNK * (p + 1), :],
                )

            # software pipeline: PE stream has no dependency gaps
            stage_load(0)
            stage_load(1)
            interleave(None, 0)
            interleave(0, 1)
            stage_load(2)
            stage_attn(0, [lambda: stage_load(3)] + pipeline_units(1, 2))
            stage_attn(1, pipeline_units(2, 3))
            stage_attn(2, pipeline_units(3, None))
            stage_attn(3)
    if not nc.is_finalized():
        nc.finalize()
    return nc


_CACHED_NC = None


def kernel(**inputs):
    global _CACHED_NC
    x = np.ascontiguousarray(np.asarray(inputs["x"], dtype=np.float32))
    WQ = np.ascontiguousarray(np.asarray(inputs["WQ"], dtype=np.float32))
    WV = np.ascontiguousarray(np.asarray(inputs["WV"], dtype=np.float32))
    B = x.shape[0]

    if _CACHED_NC is None:
        _CACHED_NC = build_nc()
    nc = _CACHED_NC

    # fp16 hi/lo limb splits (host side): hi = fp16(v), lo = fp16(v - hi)
    xh64 = x.astype(np.float16)
    xl64 = (x - xh64.astype(np.float32)).astype(np.float16)
    wfull = np.concatenate([WV, WQ], axis=1)  # [C, 2D], cols 0-63 = WV
    whh = wfull.astype(np.float16)
    wll = (wfull - whh.astype(np.float32)).astype(np.float16)
    whl_s = np.ascontiguousarray(np.stack([whh, wll], axis=1))  # [C, 2, 2D]

    in_maps = []
    for core in range(8):
        b, h = divmod(core, 2)
        cols = np.concatenate(
            [np.arange(512 * j, 512 * (j + 1)) for j in KEY_ORDER[h]]
        )
        in_maps.append(
            {
                "xth": np.ascontiguousarray(xh64[b].T[:, cols]),
                "xtl": np.ascontiguousarray(xl64[b].T[:, cols]),
                "whl": whl_s,
                "flgb": np.broadcast_to(
                    np.asarray(BIAS[h], np.float32), (P, NCHUNK)
                ).copy(),
            }
        )

    trace = os.environ.get("KERNEL_TRACE", "0") == "1"
    res = run_bass_kernel_spmd(nc, in_maps, core_ids=list(range(8)), trace=trace)
    kernel._last_results = res

    out = np.empty((B, T, D), dtype=np.float32)
    for core in range(8):
        b, h = divmod(core, 2)
        ob = res.results[core]["o"]  # [128, 16, 65]
        num = ob[:, :, :D].astype(np.float64)
        den = ob[:, :, D].astype(np.float64)
        full = (num / den[:, :, None]).astype(np.float32)  # [128, 16, 64]
        for c, j in enumerate(Q_BLOCKS[h]):
            for qb in range(NCHUNK):
                q0 = 512 * j + 128 * qb
                out[b, q0 : q0 + 128] = full[:, NCHUNK * c + qb]
    return out
